# revision 1
# baseline (speedup 1.0000x reference)
"""Causal single-head attention (B=4, S=2048, d=1024) on 8 TRN2 NeuronCores.

Sharding (uniform single program): core c -> batch b = c//2, subset
s = c%2. Per batch, the 16 query blocks of 128 rows are split into
quads t=0..3; core (b,s) owns blocks {4t+2s, 4t+2s+1}. Every core runs
the identical instruction stream (padded causal limit (t+1)*512 per
quad); the true causal boundary comes from per-core 0/1 mask tiles
supplied as input data.

K/V projections are tensor-parallel within each core pair: core (b,s)
computes the d_out-half s of kT and v for the whole batch; halves are
exchanged with a pairwise AllGather ([[0,1],[2,3],[4,5],[6,7]]).

All device inputs are host-preswizzled to partition-major [128, ...]
layouts so every DMA moves large contiguous per-partition segments.

Compute (bf16 operands, fp32 PSUM accumulate):
  P1: kT half -> kg_in, AllGather -> kt [d_out, 2048]
  P2: v half  -> vg_in, AllGather -> vv [2048, d_out]
  P0: qT = (Wq/sqrt(d)) @ xq^T -> resident SBUF (overlaps the gathers)
  P3: per quad t: scoresT[k,q] = kt.T-slices @ qT-group, p = exp(scoresT)
      (no max subtraction: |scores| <= ~2), mask, then
      out[q,:] = (pT.T @ v) / (pT.T @ ones)  -- row sums via ones-matmul.
"""
import sys

sys.path.insert(0, "/opt/trn_rl_repo")

import ml_dtypes
import numpy as np

import concourse.bass as bass  # noqa: F401
import concourse.mybir as mybir
import concourse.tile as tile
from concourse import bacc
from concourse.bass_utils import run_bass_kernel_spmd

B, S, D = 4, 2048, 1024
DC = D // 128          # 8 contraction chunks
NKB = S // 128         # 16 key blocks
SCALE = 1.0 / float(np.sqrt(D))
F32 = mybir.dt.float32
BF = mybir.dt.bfloat16
EXP = mybir.ActivationFunctionType.Exp
GROUPS = [[0, 1], [2, 3], [4, 5], [6, 7]]

_cache = {}


def build_nc():
    nc = bacc.Bacc("TRN2", target_bir_lowering=False, debug=False, num_devices=8)
    # all inputs partition-major: [128, dc, cols]
    xT = nc.dram_tensor("xT", [128, DC, S], BF, kind="ExternalInput")
    xTq = nc.dram_tensor("xTq", [128, DC, 1024], BF, kind="ExternalInput")
    WqT = nc.dram_tensor("WqT", [128, DC, D], BF, kind="ExternalInput")
    WkTh = nc.dram_tensor("WkTh", [128, DC, 512], BF, kind="ExternalInput")
    WvTh = nc.dram_tensor("WvTh", [128, DC, 512], BF, kind="ExternalInput")
    masks = nc.dram_tensor("masks", [128, 4, 256], BF, kind="ExternalInput")
    out = nc.dram_tensor("out", [1024, D], F32, kind="ExternalOutput")
    # collective buffers, partition-major SBUF images
    kg_in = nc.dram_tensor("kg_in", [128, 4, S], BF)
    kg_out = nc.dram_tensor("kg_out", [2, 128, 4, S], BF)
    vg_in = nc.dram_tensor("vg_in", [128, NKB, 512], BF)
    vg_out = nc.dram_tensor("vg_out", [2, 128, NKB, 512], BF)

    with tile.TileContext(nc) as tc:
        with (
            tc.tile_pool(name="w", bufs=1) as wp,
            tc.tile_pool(name="per", bufs=1) as per,
            tc.tile_pool(name="px", bufs=2) as pxp,
            tc.tile_pool(name="ev", bufs=6) as evp,
            tc.tile_pool(name="pt", bufs=42) as ptp,
            tc.tile_pool(name="ot", bufs=1) as otp,
            tc.tile_pool(name="sml", bufs=4) as smlp,
            tc.tile_pool(name="mix", bufs=5, space="PSUM") as mixp,
            tc.tile_pool(name="psav", bufs=3, space="PSUM") as psavp,
        ):
            # ---------------- consts + persistent ----------------
            kt = per.tile([128, DC, S], BF)        # kT: [d_out, 2048]
            vv = per.tile([128, 2, NKB, 512], BF)  # v: [2048, (rank0|rank1) 512]
            qt = per.tile([128, DC, 1024], BF)     # qT: [d_out, 1024]
            kg_sb = per.tile([128, 4, S], BF)      # K-half staging
            vg_sb = per.tile([128, NKB, 512], BF)  # V-half staging
            zeros_f = per.tile([128, 2], F32)
            ones = per.tile([128, 2], BF)
            maskt = per.tile([128, 4, 256], BF)
            nc.vector.memset(zeros_f, 0.0)
            # exp(0)=1 -> also preloads the ACT exp table long before P3
            nc.scalar.activation(ones, zeros_f, EXP)
            nc.sync.dma_start(out=maskt, in_=masks[:])

            # -------- P1: K half-projection -> kg_in --------
            wk = wp.tile([128, DC, 512], BF)
            nc.sync.dma_start(out=wk, in_=WkTh[:])
            for sc in range(4):
                xk = pxp.tile([128, DC, 512], BF, tag="xs", name=f"xk_{sc}")
                nc.sync.dma_start(out=xk, in_=xT[:, :, sc * 512:(sc + 1) * 512])
                for ocl in range(4):
                    ps = mixp.tile([128, 512], F32, tag="mix")
                    for dc in range(DC):
                        nc.tensor.matmul(
                            ps,
                            lhsT=wk[:, dc, ocl * 128:(ocl + 1) * 128],
                            rhs=xk[:, dc, :],
                            start=(dc == 0),
                            stop=(dc == DC - 1),
                        )
                    nc.vector.tensor_copy(
                        kg_sb[:, ocl, sc * 512:(sc + 1) * 512], ps
                    )

            nc.scalar.dma_start(out=kg_in[:], in_=kg_sb)
            # -------- AllGather K halves (overlaps P2/P0) --------
            nc.gpsimd.collective_compute(
                "AllGather",
                mybir.AluOpType.bypass,
                replica_groups=GROUPS,
                ins=[kg_in[:]],
                outs=[kg_out[:]],
            )

            # -------- P2: V half-projection -> vg_in --------
            wv = wp.tile([128, DC, 512], BF)
            nc.sync.dma_start(out=wv, in_=WvTh[:])
            for sc in range(4):
                xv = pxp.tile([128, DC, 512], BF, tag="xs", name=f"xv_{sc}")
                nc.sync.dma_start(out=xv, in_=xT[:, :, sc * 512:(sc + 1) * 512])
                for sb in range(4):
                    ps = mixp.tile([128, 512], F32, tag="mix", name=f"ps2_{sc}_{sb}")
                    for dc in range(DC):
                        nc.tensor.matmul(
                            ps,
                            lhsT=xv[:, dc, sb * 128:(sb + 1) * 128],
                            rhs=wv[:, dc, :],
                            start=(dc == 0),
                            stop=(dc == DC - 1),
                        )
                    nc.vector.tensor_copy(vg_sb[:, sc * 4 + sb, :], ps)

            nc.scalar.dma_start(out=vg_in[:], in_=vg_sb)
            # -------- AllGather V halves (overlaps P0) --------
            nc.gpsimd.collective_compute(
                "AllGather",
                mybir.AluOpType.bypass,
                replica_groups=GROUPS,
                ins=[vg_in[:]],
                outs=[vg_out[:]],
            )

            # -------- P0: Q projection -> qt (overlaps the gathers) --------
            wq = wp.tile([128, DC, D], BF)
            xq = wp.tile([128, DC, 1024], BF)
            nc.sync.dma_start(out=wq, in_=WqT[:])
            nc.sync.dma_start(out=xq, in_=xTq[:])
            for oc in range(8):
                pss = [
                    mixp.tile([128, 512], F32, tag="mix", name=f"ps0_{oc}_{i}")
                    for i in range(2)
                ]
                for dc in range(DC):
                    for sc in range(2):
                        nc.tensor.matmul(
                            pss[sc],
                            lhsT=wq[:, dc, oc * 128:(oc + 1) * 128],
                            rhs=xq[:, dc, sc * 512:(sc + 1) * 512],
                            start=(dc == 0),
                            stop=(dc == DC - 1),
                        )
                for sc in range(2):
                    nc.vector.tensor_copy(
                        qt[:, oc, sc * 512:(sc + 1) * 512], pss[sc]
                    )

            # -------- load gathered kt / vv (2 big DMAs each) --------
            for ch in range(2):
                for r in range(2):
                    nc.sync.dma_start(
                        out=kt[:, r * 4:(r + 1) * 4, ch * 1024:(ch + 1) * 1024],
                        in_=kg_out[r][:, :, ch * 1024:(ch + 1) * 1024],
                    )
            for r in range(2):
                nc.sync.dma_start(out=vv[:, r, :, :], in_=vg_out[r])

            # ---------------- P3: attention ----------------
            # Phase A: all scoresT + exp + mask (needs kt/qt only)
            all_pts = {}
            for t in range(4):
                L = 4 * t + 4
                for kb in range(L):
                    ps = mixp.tile([128, 512], F32, tag="mix")
                    for dc in range(DC):
                        nc.tensor.matmul(
                            ps[:, 0:256],
                            lhsT=kt[:, dc, kb * 128:(kb + 1) * 128],
                            rhs=qt[:, dc, t * 256:(t + 1) * 256],
                            start=(dc == 0),
                            stop=(dc == DC - 1),
                        )
                    pt = ptp.tile([128, 256], BF, tag="pt")
                    nc.scalar.activation(pt, ps[:, 0:256], EXP)
                    kbr = kb - 4 * t
                    if kbr >= 0:
                        nc.vector.tensor_mul(pt, pt, maskt[:, kbr, :])
                    all_pts[(t, kb)] = pt
            # Phase B: all l + av (needs vv from the second gather)
            for t in range(4):
                L = 4 * t + 4
                for j in range(2):
                    qsl = slice(j * 128, (j + 1) * 128)
                    lps = psavp.tile([128, 2], F32, tag="psav", name=f"l_{t}_{j}")
                    for kb in range(L):
                        nc.tensor.matmul(
                            lps,
                            lhsT=all_pts[(t, kb)][:, qsl],
                            rhs=ones,
                            start=(kb == 0),
                            stop=(kb == L - 1),
                        )
                    rec = smlp.tile([128, 1], F32, tag="rec")
                    nc.vector.reciprocal(rec, lps[:, 0:1])
                    ot = otp.tile([128, D], F32, tag="ot")
                    for oh in range(2):
                        avp = psavp.tile([128, 512], F32, tag="psav",
                                         name=f"av_{t}_{j}_{oh}")
                        for kb in range(L):
                            nc.tensor.matmul(
                                avp,
                                lhsT=all_pts[(t, kb)][:, qsl],
                                rhs=vv[:, oh, kb, :],
                                start=(kb == 0),
                                stop=(kb == L - 1),
                            )
                        nc.vector.tensor_scalar_mul(
                            ot[:, oh * 512:(oh + 1) * 512], avp, rec
                        )
                    nc.scalar.dma_start(
                        out=out[t * 256 + j * 128: t * 256 + (j + 1) * 128, :],
                        in_=ot,
                    )
    nc.compile()
    return nc


def _query_cols(sub):
    return np.concatenate(
        [
            np.arange((4 * t + 2 * sub) * 128, (4 * t + 2 * sub + 2) * 128)
            for t in range(4)
        ]
    )


def _masks(sub):
    m = np.zeros((4, 128, 256), np.float32)
    p = np.arange(128)[:, None]
    j = np.arange(256)[None, :]
    qoff = (2 * sub + j // 128) * 128 + j % 128
    for kbr in range(4):
        m[kbr] = (kbr * 128 + p <= qoff).astype(np.float32)
    return np.ascontiguousarray(m.transpose(1, 0, 2))  # -> [128, 4, 256]


def _pmaj(a):
    """[dc*128, cols] -> partition-major [128, dc, cols]."""
    d, cols = a.shape
    return np.ascontiguousarray(a.reshape(d // 128, 128, cols).transpose(1, 0, 2))


def kernel(x, Wq, Wk, Wv, _trace=False):
    if "nc" not in _cache:
        _cache["nc"] = build_nc()
    nc = _cache["nc"]

    bf = ml_dtypes.bfloat16
    x = np.asarray(x, dtype=np.float32)
    WqT = _pmaj((np.asarray(Wq, np.float32).T * np.float32(SCALE)).astype(bf))
    WkT = np.asarray(Wk, np.float32).T.astype(bf)
    WvT = np.asarray(Wv, np.float32).T.astype(bf)

    in_maps = []
    for c in range(8):
        b, sub = c // 2, c % 2
        xT = x[b].T.astype(bf)
        in_maps.append(
            {
                "xT": _pmaj(xT),
                "xTq": _pmaj(np.ascontiguousarray(xT[:, _query_cols(sub)])),
                "WqT": WqT,
                "WkTh": _pmaj(WkT[:, sub * 512:(sub + 1) * 512]),
                "WvTh": _pmaj(WvT[:, sub * 512:(sub + 1) * 512]),
                "masks": _masks(sub).astype(bf),
            }
        )

    res = run_bass_kernel_spmd(
        nc, in_maps, core_ids=list(range(8)), trace=_trace
    )
    full = np.empty((B, S, D), np.float32)
    for c in range(8):
        b, sub = c // 2, c % 2
        full[b, _query_cols(sub)] = res.results[c]["out"]
    if _trace:
        _cache["last_result"] = res
    return full



# revision 3
# speedup vs baseline: 2.2001x; 2.2001x over previous
"""Causal single-head attention (B=4, S=2048, d=1024) on 8 TRN2 NeuronCores.

Sharding: core c -> batch b = c//2, subset s = c%2. Per batch the 16
query blocks (128 rows) are assigned in balanced causal pairs: core
(b,s) owns pairs (lo_i, hi_i) = (2i+s, 15-2i-s), i=0..3, giving every
core 68 true causal score tiles (padded to a uniform 72). Every core
runs the identical instruction stream; causal boundaries come from
per-core 0/1 mask tiles supplied as input data.

K/V projections are tensor-parallel within each core pair: core (b,s)
computes the d_out-half s of kT (fp8, x32-scaled) and v (bf16) for the
whole batch; halves are exchanged with pairwise AllGathers
([[0,1],[2,3],[4,5],[6,7]]).

Precision: projections and AV run in bf16 (fp32 PSUM). Scores run in
fp8e4m3 DoubleRow (256-deep contraction per pass = 2x bf16 FLOPs):
host folds x32 into Wq and Wk so q' = 32q, k' = 32k sit in the fp8
sweet spot; the combined 2^15 scale is removed inside the EXP
activation (exp(s' / 32768)).

Compute:
  P1: kT half (bf16 matmul) -> fp8 kg_in, AllGather -> kt8 [d,2048] fp8
  P2: v half -> vg_in (bf16), AllGather -> vv [2048, d]
  P0: qT' -> qt8 [d, 1024] fp8 (overlaps the gathers)
  A:  per pair i: shared kb in [0,2i+2): scoresT tile [k128, q256] via
      4 DR matmuls; solo kb in [2i+2,16-2i): [k128, q128].
      p = exp(s'/32768) -> bf16, masked at causal boundary tiles.
  B:  per query chain (hi0..hi3, lo3..lo0): interleaved accumulation
      out0/out1/l over the chain's pt tiles, then out = av * (1/l).
"""
import sys

sys.path.insert(0, "/opt/trn_rl_repo")

import ml_dtypes
import numpy as np

import concourse.bass as bass  # noqa: F401
import concourse.mybir as mybir
import concourse.tile as tile
from concourse import bacc
from concourse.bass_utils import run_bass_kernel_spmd

B, S, D = 4, 2048, 1024
DC = D // 128          # 8 contraction chunks
F32 = mybir.dt.float32
BF = mybir.dt.bfloat16
E4 = mybir.dt.float8e4
E4NP = ml_dtypes.float8_e4m3
BFNP = ml_dtypes.bfloat16
DR = mybir.MatmulPerfMode.DoubleRow
EXP = mybir.ActivationFunctionType.Exp
GROUPS = [[0, 1], [2, 3], [4, 5], [6, 7]]
EXP_SCALE = 1.0 / 32768.0   # q,k both carry x32; scores carry x1024*32

_cache = {}


def build_nc():
    nc = bacc.Bacc("TRN2", target_bir_lowering=False, debug=False, num_devices=8)
    # inputs, partition-major & contiguous per planned DMA
    xT = nc.dram_tensor("xT", [128, 4, DC, 512], BF, kind="ExternalInput")
    xTq = nc.dram_tensor("xTq", [128, DC, 1024], BF, kind="ExternalInput")
    WqT = nc.dram_tensor("WqT", [128, DC, D], BF, kind="ExternalInput")
    WkTh = nc.dram_tensor("WkTh", [128, 4, DC, 128], BF, kind="ExternalInput")
    WvTh = nc.dram_tensor("WvTh", [128, DC, 512], BF, kind="ExternalInput")
    masks_sh = nc.dram_tensor("masks_sh", [128, 8, 256], BF, kind="ExternalInput")
    masks_so = nc.dram_tensor("masks_so", [128, 8, 128], BF, kind="ExternalInput")
    out = nc.dram_tensor("out", [1024, D], F32, kind="ExternalOutput")
    # collective buffers
    kg_in = nc.dram_tensor("kg_in", [128, 4, S], E4)
    kg_out = nc.dram_tensor("kg_out", [2, 128, 4, S], E4)
    vg_in = nc.dram_tensor("vg_in", [128, 16, 512], BF)
    vg_out = nc.dram_tensor("vg_out", [2, 128, 16, 512], BF)

    with tile.TileContext(nc) as tc:
        with (
            tc.tile_pool(name="w", bufs=1) as wp,
            tc.tile_pool(name="xs", bufs=1) as xsp,
            tc.tile_pool(name="per", bufs=1) as per,
            tc.tile_pool(name="pt", bufs=1) as ptp,
            tc.tile_pool(name="ot", bufs=2) as otp,
            tc.tile_pool(name="sml", bufs=4) as smlp,
            tc.tile_pool(name="mix", bufs=4, space="PSUM") as mixp,
            tc.tile_pool(name="psav", bufs=4, space="PSUM") as psavp,
        ):
            # ---------------- consts + persistent ----------------
            zeros_f = per.tile([128, 2], F32)
            ones = per.tile([128, 2], BF)
            nc.vector.memset(zeros_f, 0.0)
            # exp(0)=1 -> also preloads the ACT exp table long before A
            nc.scalar.activation(ones, zeros_f, EXP)

            # -------- P1: K half-projection (fp8 out, x32 folded) --------
            wk = [wp.tile([128, DC, 128], BF, name=f"wk_{o}") for o in range(4)]
            xs = [xsp.tile([128, DC, 512], BF, name=f"xs_{sc}") for sc in range(4)]
            nc.sync.dma_start(out=wk[0], in_=WkTh[:, 0])
            nc.sync.dma_start(out=xs[0], in_=xT[:, 0])
            for o in range(1, 4):
                nc.sync.dma_start(out=wk[o], in_=WkTh[:, o])
            for sc in range(1, 4):
                nc.sync.dma_start(out=xs[sc], in_=xT[:, sc])
            wq = wp.tile([128, DC, D], BF)
            xq = wp.tile([128, DC, 1024], BF)
            wv = wp.tile([128, DC, 512], BF)
            nc.sync.dma_start(out=wq, in_=WqT[:])
            nc.sync.dma_start(out=xq, in_=xTq[:])
            nc.sync.dma_start(out=wv, in_=WvTh[:])
            maskt_sh = per.tile([128, 8, 256], BF)
            maskt_so = per.tile([128, 8, 128], BF)
            nc.sync.dma_start(out=maskt_sh, in_=masks_sh[:])
            nc.sync.dma_start(out=maskt_so, in_=masks_so[:])

            kg_sb = per.tile([128, 4, S], E4)
            for sc in range(4):
                for ocl in range(4):
                    ps = mixp.tile([128, 512], F32, tag="mix")
                    for dc in range(DC):
                        nc.tensor.matmul(
                            ps,
                            lhsT=wk[ocl][:, dc, :],
                            rhs=xs[sc][:, dc, :],
                            start=(dc == 0),
                            stop=(dc == DC - 1),
                        )
                    nc.vector.tensor_copy(
                        kg_sb[:, ocl, sc * 512:(sc + 1) * 512], ps
                    )
            nc.scalar.dma_start(out=kg_in[:], in_=kg_sb)
            nc.gpsimd.collective_compute(
                "AllGather",
                mybir.AluOpType.bypass,
                replica_groups=GROUPS,
                ins=[kg_in[:]],
                outs=[kg_out[:]],
            )

            # -------- P2: V half-projection (bf16) --------
            vg_sb = per.tile([128, 16, 512], BF)
            for sc in range(4):
                for sb in range(4):
                    ps = mixp.tile([128, 512], F32, tag="mix", name=f"ps2_{sc}_{sb}")
                    for dc in range(DC):
                        nc.tensor.matmul(
                            ps,
                            lhsT=xs[sc][:, dc, sb * 128:(sb + 1) * 128],
                            rhs=wv[:, dc, :],
                            start=(dc == 0),
                            stop=(dc == DC - 1),
                        )
                    nc.vector.tensor_copy(vg_sb[:, sc * 4 + sb, :], ps)
            nc.scalar.dma_start(out=vg_in[:], in_=vg_sb)
            nc.gpsimd.collective_compute(
                "AllGather",
                mybir.AluOpType.bypass,
                replica_groups=GROUPS,
                ins=[vg_in[:]],
                outs=[vg_out[:]],
            )

            # -------- P0: Q projection -> qt8 (fp8, overlaps gathers) ----
            qt8 = per.tile([128, DC, 1024], E4)
            for oc in range(8):
                pss = [
                    mixp.tile([128, 512], F32, tag="mix", name=f"ps0_{oc}_{i}")
                    for i in range(2)
                ]
                for dc in range(DC):
                    for sc in range(2):
                        nc.tensor.matmul(
                            pss[sc],
                            lhsT=wq[:, dc, oc * 128:(oc + 1) * 128],
                            rhs=xq[:, dc, sc * 512:(sc + 1) * 512],
                            start=(dc == 0),
                            stop=(dc == DC - 1),
                        )
                for sc in range(2):
                    nc.vector.tensor_copy(
                        qt8[:, oc, sc * 512:(sc + 1) * 512], pss[sc]
                    )

            # -------- load gathered kt8 / vv --------
            kt8 = per.tile([128, DC, S], E4)
            vv = per.tile([128, 2, 16, 512], BF)
            for r in range(2):
                for ch in range(2):
                    nc.sync.dma_start(
                        out=kt8[:, 4 * r:4 * r + 4, ch * 1024:(ch + 1) * 1024],
                        in_=kg_out[r][:, :, ch * 1024:(ch + 1) * 1024],
                    )
                nc.sync.dma_start(out=vv[:, r], in_=vg_out[r])

            # ---------------- Phase A: scoresT + exp + mask ----------------
            pts = {}
            for i in range(4):
                qc = 256 * i
                for kb in range(0, 2 * i + 2):       # shared [k128, q256]
                    ps = mixp.tile([128, 512], F32, tag="mix")
                    for j in range(4):
                        nc.tensor.matmul(
                            ps[:, 0:256],
                            lhsT=kt8[:, 2 * j:2 * j + 2, kb * 128:(kb + 1) * 128],
                            rhs=qt8[:, 2 * j:2 * j + 2, qc:qc + 256],
                            start=(j == 0),
                            stop=(j == 3),
                            perf_mode=DR,
                        )
                    pt = ptp.tile([128, 256], BF, name=f"ptsh_{i}_{kb}")
                    nc.scalar.activation(pt, ps[:, 0:256], EXP, scale=EXP_SCALE)
                    if kb >= 2 * i:
                        nc.vector.tensor_mul(pt, pt, maskt_sh[:, kb, :])
                    pts[("sh", i, kb)] = pt
                for kb in range(2 * i + 2, 16 - 2 * i):  # solo [k128, q128]
                    ps = mixp.tile([128, 512], F32, tag="mix")
                    for j in range(4):
                        nc.tensor.matmul(
                            ps[:, 0:128],
                            lhsT=kt8[:, 2 * j:2 * j + 2, kb * 128:(kb + 1) * 128],
                            rhs=qt8[:, 2 * j:2 * j + 2, qc + 128:qc + 256],
                            start=(j == 0),
                            stop=(j == 3),
                            perf_mode=DR,
                        )
                    pt = ptp.tile([128, 128], BF, name=f"ptso_{i}_{kb}")
                    nc.scalar.activation(pt, ps[:, 0:128], EXP, scale=EXP_SCALE)
                    if kb >= 14 - 2 * i:
                        nc.vector.tensor_mul(
                            pt, pt, maskt_so[:, kb - 14 + 2 * i + 2 * i, :]
                        )
                    pts[("so", i, kb)] = pt

            # ---------------- Phase B: chains ----------------
            chains = []
            for i in range(4):  # hi chains, longest first
                tiles = [(("sh", i, kb), slice(128, 256), kb)
                         for kb in range(0, 2 * i + 2)]
                tiles += [(("so", i, kb), slice(0, 128), kb)
                          for kb in range(2 * i + 2, 16 - 2 * i)]
                chains.append(tiles)
            for i in (3, 2, 1, 0):  # lo chains, shortest last
                chains.append([(("sh", i, kb), slice(0, 128), kb)
                               for kb in range(0, 2 * i + 2)])

            for ci, tiles in enumerate(chains):
                avs = [
                    psavp.tile([128, 512], F32, tag="psav", name=f"av_{ci}_{oh}")
                    for oh in range(2)
                ]
                lps = psavp.tile([128, 2], F32, tag="psav", name=f"l_{ci}")
                n = len(tiles)
                for idx, (key, qsl, kb) in enumerate(tiles):
                    pt = pts[key]
                    first, last = idx == 0, idx == n - 1
                    for oh in range(2):
                        nc.tensor.matmul(
                            avs[oh],
                            lhsT=pt[:, qsl],
                            rhs=vv[:, oh, kb, :],
                            start=first,
                            stop=last,
                        )
                    nc.tensor.matmul(
                        lps, lhsT=pt[:, qsl], rhs=ones, start=first, stop=last
                    )
                rec = smlp.tile([128, 1], F32, tag="rec")
                nc.vector.reciprocal(rec, lps[:, 0:1])
                ot = otp.tile([128, D], F32, tag="ot")
                for oh in range(2):
                    nc.vector.tensor_scalar_mul(
                        ot[:, oh * 512:(oh + 1) * 512], avs[oh], rec
                    )
                nc.scalar.dma_start(
                    out=out[ci * 128:(ci + 1) * 128, :], in_=ot
                )
    nc.compile()
    return nc


def _pair_blocks(sub):
    """(lo_i, hi_i) query-block ids for pairs i=0..3."""
    return [(2 * i + sub, 15 - 2 * i - sub) for i in range(4)]


def _query_cols(sub):
    """qt/xTq column order: [lo0, hi0, lo1, hi1, ...] x 128 each."""
    cols = []
    for lo, hi in _pair_blocks(sub):
        cols.append(np.arange(lo * 128, lo * 128 + 128))
        cols.append(np.arange(hi * 128, hi * 128 + 128))
    return np.concatenate(cols)


def _chain_blocks(sub):
    """Output row order: chains hi0..hi3 then lo3..lo0."""
    pb = _pair_blocks(sub)
    return [pb[i][1] for i in range(4)] + [pb[i][0] for i in (3, 2, 1, 0)]


def _masks(sub):
    """masks_sh [128, 8, 256] and masks_so [128, 8, 128] (slot = 2i+d)."""
    p = np.arange(128)[:, None]
    j = np.arange(128)[None, :]
    msh = np.zeros((8, 128, 256), np.float32)
    mso = np.zeros((8, 128, 128), np.float32)
    for i, (lo, hi) in enumerate(_pair_blocks(sub)):
        for d in range(2):
            kb = 2 * i + d
            msh[2 * i + d, :, 0:128] = (kb * 128 + p <= lo * 128 + j)
            msh[2 * i + d, :, 128:256] = (kb * 128 + p <= hi * 128 + j)
            kbs = 14 - 2 * i + d
            mso[2 * i + d] = (kbs * 128 + p <= hi * 128 + j)
    return (
        np.ascontiguousarray(msh.transpose(1, 0, 2)),
        np.ascontiguousarray(mso.transpose(1, 0, 2)),
    )


def _pmaj(a):
    """[dc*128, cols] -> partition-major [128, dc, cols]."""
    d, cols = a.shape
    return np.ascontiguousarray(a.reshape(d // 128, 128, cols).transpose(1, 0, 2))


def kernel(x, Wq, Wk, Wv, _trace=False):
    if "nc" not in _cache:
        _cache["nc"] = build_nc()
    nc = _cache["nc"]

    x = np.asarray(x, dtype=np.float32)
    # q' = 32q, k' = 32k: x32 into Wq (net of the folded 1/sqrt(d)) and Wk;
    # scores then carry 32*32*32 = 2^15, removed by EXP_SCALE.
    WqTs = _pmaj((np.asarray(Wq, np.float32).T * np.float32(32.0)).astype(BFNP))
    WkTs = (np.asarray(Wk, np.float32).T * np.float32(32.0)).astype(BFNP)
    WvT = np.asarray(Wv, np.float32).T.astype(BFNP)

    in_maps = []
    for c in range(8):
        b, sub = c // 2, c % 2
        xTb = x[b].T.astype(BFNP)                      # [1024, 2048]
        xTp = _pmaj(xTb)                               # [128, 8, 2048]
        xT4 = np.ascontiguousarray(
            xTp.reshape(128, DC, 4, 512).transpose(0, 2, 1, 3)
        )                                              # [128, 4, 8, 512]
        wkh = _pmaj(WkTs[:, sub * 512:(sub + 1) * 512])  # [128, 8, 512]
        wk4 = np.ascontiguousarray(
            wkh.reshape(128, DC, 4, 128).transpose(0, 2, 1, 3)
        )                                              # [128, 4, 8, 128]
        msh, mso = _masks(sub)
        in_maps.append(
            {
                "xT": xT4,
                "xTq": _pmaj(np.ascontiguousarray(xTb[:, _query_cols(sub)])),
                "WqT": WqTs,
                "WkTh": wk4,
                "WvTh": _pmaj(WvT[:, sub * 512:(sub + 1) * 512]),
                "masks_sh": msh.astype(BFNP),
                "masks_so": mso.astype(BFNP),
            }
        )

    res = run_bass_kernel_spmd(
        nc, in_maps, core_ids=list(range(8)), trace=_trace
    )
    full = np.empty((B, S, D), np.float32)
    for c in range(8):
        b, sub = c // 2, c % 2
        for pos, qb in enumerate(_chain_blocks(sub)):
            full[b, qb * 128:(qb + 1) * 128] = (
                res.results[c]["out"][pos * 128:(pos + 1) * 128]
            )
    if _trace:
        _cache["last_result"] = res
    return full
